# revision 3
# baseline (speedup 1.0000x reference)
"""TSM-style 3-tap depthwise temporal conv on 8 Trainium2 NeuronCores.

out[n, t, c, h, w] = w[c,0]*x[n,t-1,c,h,w] + w[c,1]*x[n,t,c,h,w]
                   + w[c,2]*x[n,t+1,c,h,w]   (zero-padded at clip edges)

Sharding: pure data parallel over the nt (clip-batch) axis — each of the 8
cores gets whole clips (nt=64, n_segment=8 -> one 8-frame clip per core).
Weight (c,3) is replicated.

Platform model (measured on this axon/trn2 virtualized stack): execution is
dominated by a large per-instruction cost (~40-60us dispatch + size-dependent
part), with only limited DMA/compute overlap; standalone semaphore
instructions cost as much as compute. The kernel therefore:

  - is written in *raw bacc* (nc.Block) rather than the Tile framework, with
    every semaphore inc attached to a data instruction via .then_inc (the
    Tile scheduler emits ~4 standalone EventSemaphore instructions per pass,
    each carrying full instruction-dispatch cost);
  - uses the minimal instruction count: 2 loads (12.8MB fp32 each; x for one
    128-channel block fills a [128, 8, 3136] SBUF tile), 6 DVE ops
    (tensor_scalar_mul + 2 scalar_tensor_tensor per block), and ONE merged
    store of the whole output;
  - stores the output in bf16: the y tile for BOTH channel blocks fits in
    SBUF ([128, 8, 2, 3136] bf16, f-major block-inner so the single store is
    a uniform-stride 3D DMA), halving store bytes and speeding the DVE
    accumulation ops (bf16 y: ~1.2ns/elem vs 2.4 fp32). Output rel-err vs
    fp32 is ~4e-3 (3 bf16 roundings), well inside the 2e-2 gate.

Measured: ~606us/pass at K=422 chain scale vs 780us for the same raw fp32
structure and ~950us for the original Tile-framework baseline.
"""

import contextlib

import numpy as np

import concourse.bacc as bacc
import concourse.mybir as mybir
from concourse.bass_utils import run_bass_kernel_spmd

N_CORES = 8
P = 128  # SBUF partitions

FP = mybir.dt.float32
BF = mybir.dt.bfloat16
MULT = mybir.AluOpType.mult
ADD = mybir.AluOpType.add

_cache = {}


def emit_conv_raw(nc, block, sems, tiles, src, dst, F, C, n_seg, repeat=1,
                  chain=False, ydt=BF):
    """Emit `repeat` conv passes src->dst in raw bacc with attached sems.

    src/dst: callables k -> (DRAM handle); dst dtype must equal ydt.
    tiles: (wt, xt, yb); sems: (semW, semF, semL, semC, semS).
    chain=True adds cross-pass WAR/RAW waits for the timing harness
    (scratch ping-pong); the real kernel uses repeat=1, chain=False.
    """
    semW, semF, semL, semC, semS = sems
    wt, xt, yb = tiles
    NB = C // P
    HW = xt.shape[2]
    n_clips = max(F // n_seg, 1)
    S = min(n_seg, F)
    ys = [yb[:, :, b, :] for b in range(NB)]

    def src_view(k, b):
        return src(k)[:, b * P:(b + 1) * P, :].rearrange("f c x -> c f x")

    def dst_view_big(k):
        return dst(k).rearrange("f (b c) x -> c (f b) x", c=P)

    def loads(eng):
        if chain:
            eng.wait_ge(semF, 16 * NB + 1)  # scratch fill done
        for k in range(repeat):
            for b in range(NB):
                i = NB * k + b
                if i > 0:
                    eng.wait_ge(semC, i)  # compute i-1 consumed shared xt
                eng.dma_start(xt[:, :, :], src_view(k, b)).then_inc(semL, 16)

    def stores(eng):
        for k in range(repeat):
            eng.wait_ge(semC, NB * (k + 1))
            eng.dma_start(dst_view_big(k),
                          yb.rearrange("c f b x -> c (f b) x")).then_inc(semS, 16)

    def compute(eng):
        for k in range(repeat):
            for b in range(NB):
                i = NB * k + b
                w0 = wt[:, b, 0:1]
                w1 = wt[:, b, 1:2]
                w2 = wt[:, b, 2:3]
                y_ = ys[b]
                # waits ride on the first compute instruction of the block —
                # standalone wait_ge emits an EventSemaphore instruction on
                # DVE, which costs a full instruction dispatch on this stack
                ts = eng.tensor_scalar_mul(y_, xt[:, :, :], w1)
                if i == 0:
                    ts.wait_op(semW, 16, "sem-ge")
                ts.wait_op(semL, 16 * (i + 1), "sem-ge")
                if chain and b == 0 and k > 0:
                    ts.wait_op(semS, 16 * k, "sem-ge")  # store k-1 read yb
                for c in range(n_clips):
                    lo, hi = c * S, (c + 1) * S
                    eng.scalar_tensor_tensor(
                        y_[:, lo + 1:hi, :], xt[:, lo:hi - 1, :], w0,
                        y_[:, lo + 1:hi, :], MULT, ADD)
                    last = eng.scalar_tensor_tensor(
                        y_[:, lo:hi - 1, :], xt[:, lo + 1:hi, :], w2,
                        y_[:, lo:hi - 1, :], MULT, ADD)
                last.then_inc(semC, 1)

    return loads, stores, compute


def _build(F, C, HW, n_seg, ydt=BF):
    """Single-pass program: x (F, C, HW) f32 -> out (F, C, HW) ydt."""
    nc = bacc.Bacc("TRN2", target_bir_lowering=False, debug=False,
                   num_devices=N_CORES)
    x = nc.dram_tensor("x", (F, C, HW), FP, kind="ExternalInput")
    w = nc.dram_tensor("weight", (C, 3), FP, kind="ExternalInput")
    out = nc.dram_tensor("out", (F, C, HW), ydt, kind="ExternalOutput")
    NB = C // P

    stack = contextlib.ExitStack()
    block = stack.enter_context(nc.Block())
    sems = tuple(stack.enter_context(nc.semaphore(s))
                 for s in ("semW", "semF", "semL", "semC", "semS"))
    wt = stack.enter_context(nc.sbuf_tensor("wt", [P, NB, 3], FP))
    xt = stack.enter_context(nc.sbuf_tensor("xt", [P, F, HW], FP))
    yb = stack.enter_context(nc.sbuf_tensor("yb", [P, F, NB, HW], ydt))
    semW = sems[0]

    loads, stores, compute = emit_conv_raw(
        nc, block, sems, (wt, xt, yb), lambda k: x, lambda k: out,
        F, C, n_seg, repeat=1, chain=False, ydt=ydt)

    def sync_body(eng):
        eng.dma_start(wt[:, :, :],
                      w.ap().rearrange("(b c) k -> c b k", c=P)).then_inc(semW, 16)
        loads(eng)

    block.sync(sync_body)
    block.scalar(stores)
    block.vector(compute)
    stack.close()
    nc.compile()
    return nc


def _get_program(F, C, HW, n_seg):
    key = (F, C, HW, n_seg)
    if key not in _cache:
        _cache[key] = _build(F, C, HW, n_seg)
    return _cache[key]


def kernel(x, weight, n_segment, **_kw):
    x = np.asarray(x)
    weight = np.ascontiguousarray(np.asarray(weight, dtype=np.float32))
    n_seg = int(np.asarray(n_segment))
    nt, C, H, W = x.shape
    HW = H * W
    assert nt % N_CORES == 0
    F = nt // N_CORES
    # each core must hold whole clips
    assert F % n_seg == 0 or n_seg % F == 0, (F, n_seg)
    assert C % P == 0, C

    nc = _get_program(F, C, HW, n_seg)

    xs = np.ascontiguousarray(x, dtype=np.float32).reshape(nt, C, HW)
    in_maps = [
        {"x": xs[i * F:(i + 1) * F], "weight": weight} for i in range(N_CORES)
    ]
    res = run_bass_kernel_spmd(nc, in_maps, list(range(N_CORES)))
    out = np.concatenate(
        [np.asarray(res.results[i]["out"], dtype=np.float32)
         for i in range(N_CORES)], axis=0)
    return out.reshape(nt, C, H, W).astype(np.float32, copy=False)


# revision 6
# speedup vs baseline: 1.1161x; 1.1161x over previous
"""TSM-style 3-tap depthwise temporal conv on 8 Trainium2 NeuronCores.

out[n, t, c, h, w] = w[c,0]*x[n,t-1,c,h,w] + w[c,1]*x[n,t,c,h,w]
                   + w[c,2]*x[n,t+1,c,h,w]   (zero-padded at clip edges)

Sharding: pure data parallel over the nt (clip-batch) axis — each of the 8
cores gets whole clips (nt=64, n_segment=8 -> one 8-frame clip per core).
Weight (c,3) is replicated.

Platform model (measured on this axon/trn2 virtualized stack): execution is
dominated by a large per-instruction cost (~40-60us dispatch + size-dependent
part), with only limited DMA/compute overlap; standalone semaphore
instructions cost as much as compute. The kernel therefore:

  - is written in *raw bacc* (nc.Block) rather than the Tile framework, with
    every semaphore inc attached to a data instruction via .then_inc (the
    Tile scheduler emits ~4 standalone EventSemaphore instructions per pass,
    each carrying full instruction-dispatch cost);
  - uses the minimal instruction count: 2 loads (12.8MB fp32 each; x for one
    128-channel block fills a [128, 8, 3136] SBUF tile), 6 DVE ops
    (tensor_scalar_mul + 2 scalar_tensor_tensor per block), and ONE merged
    store of the whole output;
  - stores the output in bf16: the y tile for BOTH channel blocks fits in
    SBUF ([128, 8, 2, 3136] bf16, f-major block-inner so the single store is
    a uniform-stride 3D DMA), halving store bytes and speeding the DVE
    accumulation ops (bf16 y: ~1.2ns/elem vs 2.4 fp32). Output rel-err vs
    fp32 is ~4e-3 (3 bf16 roundings), well inside the 2e-2 gate.

Measured: ~606us/pass at K=422 chain scale vs 780us for the same raw fp32
structure and ~950us for the original Tile-framework baseline.
"""

import contextlib

import numpy as np

import concourse.bacc as bacc
import concourse.mybir as mybir
from concourse.bass_utils import run_bass_kernel_spmd

N_CORES = 8
P = 128  # SBUF partitions

FP = mybir.dt.float32
BF = mybir.dt.bfloat16
MULT = mybir.AluOpType.mult
ADD = mybir.AluOpType.add

_cache = {}


def emit_conv_raw(nc, block, sems, tiles, src, dst, F, C, n_seg, repeat=1,
                  chain=False, ydt=BF):
    """Emit `repeat` conv passes src->dst in raw bacc with attached sems.

    src/dst: callables k -> (DRAM handle); dst dtype must equal ydt.
    tiles: (wt, xt, yb); sems: (semW, semF, semL, semC, semS).
    chain=True adds cross-pass WAR/RAW waits for the timing harness
    (scratch ping-pong); the real kernel uses repeat=1, chain=False.
    """
    semF, semL, semC, semS = sems
    wt, xt, yb = tiles
    NB = C // P
    HW = xt.shape[2]
    n_clips = max(F // n_seg, 1)
    S = min(n_seg, F)
    ys = [yb[:, :, b, :] for b in range(NB)]

    def src_view(k, b):
        return src(k)[:, b * P:(b + 1) * P, :].rearrange("f c x -> c f x")

    def dst_view_big(k):
        return dst(k).rearrange("f (b c) x -> c (f b) x", c=P)

    def loads(eng):
        if chain:
            eng.wait_ge(semF, 16 * NB + 1)  # scratch fill done
        for k in range(repeat):
            for b in range(NB):
                i = NB * k + b
                if i > 0:
                    eng.wait_ge(semC, i)  # compute i-1 consumed shared xt
                if chain and b == 0 and k > 0:
                    # yb WAR moved load-side: ts of pass k waits this load,
                    # which waits store k-1 (DVE instructions hold only one
                    # attached wait; engine-level waits fuse into the DMA)
                    eng.wait_ge(semS, 16 * k)
                eng.dma_start(xt[:, :, :], src_view(k, b)).then_inc(semL, 16)

    def stores(eng):
        for k in range(repeat):
            eng.wait_ge(semC, NB * (k + 1))
            eng.dma_start(dst_view_big(k),
                          yb.rearrange("c f b x -> c (f b) x")).then_inc(semS, 16)

    def compute(eng):
        for k in range(repeat):
            for b in range(NB):
                i = NB * k + b
                w0 = wt[:, b, 0:1]
                w1 = wt[:, b, 1:2]
                w2 = wt[:, b, 2:3]
                y_ = ys[b]
                # the single allowed wait rides on the block's first compute
                # instruction — standalone wait_ge emits an EventSemaphore
                # instruction on DVE, costing a full instruction dispatch.
                # semL counts the wtile load too (16*(i+2) = wtile + loads
                # 0..i done), so no separate weight wait is needed.
                ts = eng.tensor_scalar_mul(y_, xt[:, :, :], w1)
                ts.wait_op(semL, 16 * (i + 2), "sem-ge")
                for c in range(n_clips):
                    lo, hi = c * S, (c + 1) * S
                    eng.scalar_tensor_tensor(
                        y_[:, lo + 1:hi, :], xt[:, lo:hi - 1, :], w0,
                        y_[:, lo + 1:hi, :], MULT, ADD)
                    last = eng.scalar_tensor_tensor(
                        y_[:, lo:hi - 1, :], xt[:, lo + 1:hi, :], w2,
                        y_[:, lo:hi - 1, :], MULT, ADD)
                last.then_inc(semC, 1)

    return loads, stores, compute


def _build(F, C, HW, n_seg, ydt=BF):
    """Single-pass program: x (F, C, HW) f32 -> out (F, C, HW) ydt."""
    nc = bacc.Bacc("TRN2", target_bir_lowering=False, debug=False,
                   num_devices=N_CORES)
    x = nc.dram_tensor("x", (F, C, HW), FP, kind="ExternalInput")
    w = nc.dram_tensor("weight", (C, 3), FP, kind="ExternalInput")
    out = nc.dram_tensor("out", (F, C, HW), ydt, kind="ExternalOutput")
    NB = C // P

    stack = contextlib.ExitStack()
    block = stack.enter_context(nc.Block())
    sems = tuple(stack.enter_context(nc.semaphore(s))
                 for s in ("semF", "semL", "semC", "semS"))
    wt = stack.enter_context(nc.sbuf_tensor("wt", [P, NB, 3], FP))
    xt = stack.enter_context(nc.sbuf_tensor("xt", [P, F, HW], FP))
    yb = stack.enter_context(nc.sbuf_tensor("yb", [P, F, NB, HW], ydt))
    semL = sems[1]

    loads, stores, compute = emit_conv_raw(
        nc, block, sems, (wt, xt, yb), lambda k: x, lambda k: out,
        F, C, n_seg, repeat=1, chain=False, ydt=ydt)

    def sync_body(eng):
        eng.dma_start(wt[:, :, :],
                      w.ap().rearrange("(b c) k -> c b k", c=P)).then_inc(semL, 16)
        loads(eng)

    block.sync(sync_body)
    block.scalar(stores)
    block.vector(compute)
    stack.close()
    nc.compile()
    return nc


def _get_program(F, C, HW, n_seg):
    key = (F, C, HW, n_seg)
    if key not in _cache:
        _cache[key] = _build(F, C, HW, n_seg)
    return _cache[key]


def kernel(x, weight, n_segment, **_kw):
    x = np.asarray(x)
    weight = np.ascontiguousarray(np.asarray(weight, dtype=np.float32))
    n_seg = int(np.asarray(n_segment))
    nt, C, H, W = x.shape
    HW = H * W
    assert nt % N_CORES == 0
    F = nt // N_CORES
    # each core must hold whole clips
    assert F % n_seg == 0 or n_seg % F == 0, (F, n_seg)
    assert C % P == 0, C

    nc = _get_program(F, C, HW, n_seg)

    xs = np.ascontiguousarray(x, dtype=np.float32).reshape(nt, C, HW)
    in_maps = [
        {"x": xs[i * F:(i + 1) * F], "weight": weight} for i in range(N_CORES)
    ]
    res = run_bass_kernel_spmd(nc, in_maps, list(range(N_CORES)))
    out = np.concatenate(
        [np.asarray(res.results[i]["out"], dtype=np.float32)
         for i in range(N_CORES)], axis=0)
    return out.reshape(nt, C, H, W).astype(np.float32, copy=False)


# revision 8
# speedup vs baseline: 1.1287x; 1.0113x over previous
"""TSM-style 3-tap depthwise temporal conv on 8 Trainium2 NeuronCores.

out[n, t, c, h, w] = w[c,0]*x[n,t-1,c,h,w] + w[c,1]*x[n,t,c,h,w]
                   + w[c,2]*x[n,t+1,c,h,w]   (zero-padded at clip edges)

Sharding: pure data parallel over the nt (clip-batch) axis — each of the 8
cores gets whole clips (nt=64, n_segment=8 -> one 8-frame clip per core).
Weight (c,3) is replicated.

Platform model (measured on this axon/trn2 virtualized stack): execution is
dominated by a large per-instruction dispatch cost (~40-60us plus a
size-dependent part), with limited engine/DMA overlap; standalone semaphore
instructions cost as much as compute ops. Design consequences:

  - raw bacc (nc.Block) instead of the Tile framework: every semaphore inc
    is attached to a data instruction via .then_inc and every DVE wait rides
    on a compute instruction via .wait_op (the Tile scheduler emits ~4
    standalone EventSemaphore instructions per pass, each costing a full
    dispatch). Note: an instruction holds at most ONE attached wait, and
    every DMA must carry a sem update or walrus crashes.
  - minimal instruction count (9 per pass): 2 casting loads (fp32 DRAM ->
    bf16 SBUF via gpsimd SWDGE, 12.8MB read each) into SEPARATE x tiles so
    the block-B load overlaps block-A compute; 6 DVE ops (tensor_scalar_mul
    + 2 scalar_tensor_tensor per 128-channel block, all-bf16 operands with
    fp32 per-partition weight scalars); ONE merged store of the whole
    output in bf16 ([128, 8, 2, 3136] f-major block-inner y tile makes the
    store a uniform-stride 3D DMA).
  - bf16 x and y: rel err vs the fp32 reference is ~9e-3 (input rounding +
    3 output roundings), inside the 2e-2 gate; halves store bytes and
    speeds DVE accumulation.

Measured (k=2/122 repeat-chain differencing): ~480-580us/pass vs 697us for
the Tile-framework fp32 baseline.
"""

import contextlib

import numpy as np

import concourse.bacc as bacc
import concourse.mybir as mybir
from concourse.bass_utils import run_bass_kernel_spmd

N_CORES = 8
P = 128  # SBUF partitions

FP = mybir.dt.float32
BF = mybir.dt.bfloat16
MULT = mybir.AluOpType.mult
ADD = mybir.AluOpType.add

_cache = {}


def emit_conv_raw(nc, w, sems, tiles, src, dst, F, C, n_seg, repeat=1,
                  chain=False):
    """Emit bodies for `repeat` conv passes src->dst (raw bacc, bf16 x/y).

    Returns (loads_body, stores_body, compute_body) closures for the
    gpsimd / scalar / vector engines. src/dst: callables k -> DRAM handle
    (src fp32, dst bf16). tiles: (wt, xA, xB, yb). chain=True adds the
    cross-pass waits used by the timing harness's scratch chain; the real
    kernel uses repeat=1, chain=False.
    """
    semF, semL, semC, semS = sems
    wt, xA, xB, yb = tiles
    NB = C // P
    HW = xA.shape[2]
    n_clips = max(F // n_seg, 1)
    S = min(n_seg, F)
    xs = [xA, xB]
    ys = [yb[:, :, b, :] for b in range(NB)]

    def src_view(k, b):
        return src(k)[:, b * P:(b + 1) * P, :].rearrange("f c x -> c f x")

    def loads(eng):
        # gpsimd SWDGE: fp32 DRAM -> bf16 SBUF casting loads
        eng.dma_start(wt[:, :, :],
                      w.ap().rearrange("(b c) k -> c b k", c=P)).then_inc(semL, 16)
        if chain:
            eng.wait_ge(semF, 16 * 2 * 16 + 1)  # scratch fill done
        for k in range(repeat):
            for b in range(NB):
                if chain and k > 0:
                    # x[b] WAR: compute (k-1, b) consumed it
                    eng.wait_ge(semC, NB * (k - 1) + b + 1)
                    if b == 0:
                        # yb WAR: store k-1 done reading yb; ts of pass k
                        # waits this load, which waits the store (a DVE
                        # instruction holds only one attached wait)
                        eng.wait_ge(semS, 16 * k)
                eng.dma_start(xs[b][:, :, :], src_view(k, b)).then_inc(semL, 16)

    def stores(eng):
        for k in range(repeat):
            eng.wait_ge(semC, NB * (k + 1))
            eng.dma_start(dst(k).rearrange("f (b c) x -> c (f b) x", c=P),
                          yb.rearrange("c f b x -> c (f b) x")).then_inc(semS, 16)

    def compute(eng):
        for k in range(repeat):
            for b in range(NB):
                i = NB * k + b
                w0 = wt[:, b, 0:1]
                w1 = wt[:, b, 1:2]
                w2 = wt[:, b, 2:3]
                y_ = ys[b]
                x_ = xs[b]
                # the single allowed attached wait rides on the block's
                # first compute op; semL counts the wtile load too, so
                # 16*(i+2) = wtile + loads 0..i complete
                ts = eng.tensor_scalar_mul(y_, x_[:, :, :], w1)
                ts.wait_op(semL, 16 * (i + 2), "sem-ge")
                for c in range(n_clips):
                    lo, hi = c * S, (c + 1) * S
                    eng.scalar_tensor_tensor(
                        y_[:, lo + 1:hi, :], x_[:, lo:hi - 1, :], w0,
                        y_[:, lo + 1:hi, :], MULT, ADD)
                    last = eng.scalar_tensor_tensor(
                        y_[:, lo:hi - 1, :], x_[:, lo + 1:hi, :], w2,
                        y_[:, lo:hi - 1, :], MULT, ADD)
                last.then_inc(semC, 1)

    return loads, stores, compute


def _build(F, C, HW, n_seg):
    """Single-pass program: x (F, C, HW) f32 -> out (F, C, HW) bf16."""
    nc = bacc.Bacc("TRN2", target_bir_lowering=False, debug=False,
                   num_devices=N_CORES)
    x = nc.dram_tensor("x", (F, C, HW), FP, kind="ExternalInput")
    w = nc.dram_tensor("weight", (C, 3), FP, kind="ExternalInput")
    out = nc.dram_tensor("out", (F, C, HW), BF, kind="ExternalOutput")
    NB = C // P

    stack = contextlib.ExitStack()
    block = stack.enter_context(nc.Block())
    sems = tuple(stack.enter_context(nc.semaphore(s))
                 for s in ("semF", "semL", "semC", "semS"))
    wt = stack.enter_context(nc.sbuf_tensor("wt", [P, NB, 3], FP))
    xA = stack.enter_context(nc.sbuf_tensor("xA", [P, F, HW], BF))
    xB = stack.enter_context(nc.sbuf_tensor("xB", [P, F, HW], BF))
    yb = stack.enter_context(nc.sbuf_tensor("yb", [P, F, NB, HW], BF))

    loads, stores, compute = emit_conv_raw(
        nc, w, sems, (wt, xA, xB, yb), lambda k: x, lambda k: out,
        F, C, n_seg, repeat=1, chain=False)

    block.gpsimd(loads)
    block.scalar(stores)
    block.vector(compute)
    stack.close()
    nc.compile()
    return nc


def _get_program(F, C, HW, n_seg):
    key = (F, C, HW, n_seg)
    if key not in _cache:
        _cache[key] = _build(F, C, HW, n_seg)
    return _cache[key]


def kernel(x, weight, n_segment, **_kw):
    x = np.asarray(x)
    weight = np.ascontiguousarray(np.asarray(weight, dtype=np.float32))
    n_seg = int(np.asarray(n_segment))
    nt, C, H, W = x.shape
    HW = H * W
    assert nt % N_CORES == 0
    F = nt // N_CORES
    # each core must hold whole clips
    assert F % n_seg == 0 or n_seg % F == 0, (F, n_seg)
    assert C % P == 0, C

    nc = _get_program(F, C, HW, n_seg)

    xs = np.ascontiguousarray(x, dtype=np.float32).reshape(nt, C, HW)
    in_maps = [
        {"x": xs[i * F:(i + 1) * F], "weight": weight} for i in range(N_CORES)
    ]
    res = run_bass_kernel_spmd(nc, in_maps, list(range(N_CORES)))
    out = np.concatenate(
        [np.asarray(res.results[i]["out"], dtype=np.float32)
         for i in range(N_CORES)], axis=0)
    return out.reshape(nt, C, H, W).astype(np.float32, copy=False)


# revision 9
# speedup vs baseline: 1.3088x; 1.1595x over previous
"""TSM-style 3-tap depthwise temporal conv on 8 Trainium2 NeuronCores.

out[n, t, c, h, w] = w[c,0]*x[n,t-1,c,h,w] + w[c,1]*x[n,t,c,h,w]
                   + w[c,2]*x[n,t+1,c,h,w]   (zero-padded at clip edges)

Sharding: pure data parallel over the nt (clip-batch) axis — each of the 8
cores gets whole clips (nt=64, n_segment=8 -> one 8-frame clip per core).
Weight (c,3) is replicated.

Platform model (measured on this axon/trn2 virtualized stack): execution is
dominated by a large per-instruction dispatch cost (~40-60us plus a
size-dependent part), with limited engine/DMA overlap; standalone semaphore
instructions cost as much as compute ops. Design consequences:

  - raw bacc (nc.Block) instead of the Tile framework: every semaphore inc
    is attached to a data instruction via .then_inc and every DVE wait rides
    on a compute instruction via .wait_op (the Tile scheduler emits ~4
    standalone EventSemaphore instructions per pass, each costing a full
    dispatch). Note: an instruction holds at most ONE attached wait, and
    every DMA must carry a sem update or walrus crashes.
  - minimal instruction count (10 per pass): 2 casting loads (fp32 DRAM ->
    bf16 SBUF via gpsimd SWDGE, 12.8MB read each) into SEPARATE x tiles so
    the block-B load overlaps block-A compute; 6 DVE ops (tensor_scalar_mul
    + 2 scalar_tensor_tensor per 128-channel block, all-bf16 operands with
    fp32 per-partition weight scalars); 2 per-block bf16 stores, so store A
    overlaps compute B and the steady-state cycle (compute -> store -> next
    load on the same buffers) is per-block rather than whole-pass.
  - bf16 x and y: rel err vs the fp32 reference is ~9e-3 (input rounding +
    3 output roundings), inside the 2e-2 gate; halves store bytes and
    speeds DVE accumulation.

Measured (k=2/122 repeat-chain differencing): ~480-580us/pass vs 697us for
the Tile-framework fp32 baseline.
"""

import contextlib

import numpy as np

import concourse.bacc as bacc
import concourse.mybir as mybir
from concourse.bass_utils import run_bass_kernel_spmd

N_CORES = 8
P = 128  # SBUF partitions

FP = mybir.dt.float32
BF = mybir.dt.bfloat16
MULT = mybir.AluOpType.mult
ADD = mybir.AluOpType.add

_cache = {}


def emit_conv_raw(nc, w, sems, tiles, src, dst, F, C, n_seg, repeat=1,
                  chain=False):
    """Emit bodies for `repeat` conv passes src->dst (raw bacc, bf16 x/y).

    Returns (loads_body, stores_body, compute_body) closures for the
    gpsimd / scalar / vector engines. src/dst: callables k -> DRAM handle
    (src fp32, dst bf16). tiles: (wt, xA, xB, yb). chain=True adds the
    cross-pass waits used by the timing harness's scratch chain; the real
    kernel uses repeat=1, chain=False.
    """
    semF, semL, semC, semS = sems
    wt, xA, xB, yb = tiles
    NB = C // P
    HW = xA.shape[2]
    n_clips = max(F // n_seg, 1)
    S = min(n_seg, F)
    xs = [xA, xB]
    ys = [yb[:, :, b, :] for b in range(NB)]

    def src_view(k, b):
        return src(k)[:, b * P:(b + 1) * P, :].rearrange("f c x -> c f x")

    def loads(eng):
        # gpsimd SWDGE: fp32 DRAM -> bf16 SBUF casting loads
        eng.dma_start(wt[:, :, :],
                      w.ap().rearrange("(b c) k -> c b k", c=P)).then_inc(semL, 16)
        if chain:
            eng.wait_ge(semF, 16 * 2 * 16 + 1)  # scratch fill done
        for k in range(repeat):
            for b in range(NB):
                if chain and k > 0:
                    # x[b] WAR: compute (k-1, b) consumed it
                    eng.wait_ge(semC, NB * (k - 1) + b + 1)
                    # y[b] WAR: store (k-1, b) done reading this block's y;
                    # ts of pass k waits this load, which waits that store
                    # (a DVE instruction holds only one attached wait)
                    eng.wait_ge(semS, 16 * (NB * (k - 1) + b + 1))
                eng.dma_start(xs[b][:, :, :], src_view(k, b)).then_inc(semL, 16)

    def stores(eng):
        # per-block stores: store b=0 runs concurrently with block-1 compute,
        # and next-pass loads wait only their own block's store
        for k in range(repeat):
            for b in range(NB):
                eng.wait_ge(semC, NB * k + b + 1)
                eng.dma_start(
                    dst(k)[:, b * P:(b + 1) * P, :].rearrange("f c x -> c f x"),
                    ys[b]).then_inc(semS, 16)

    def compute(eng):
        for k in range(repeat):
            for b in range(NB):
                i = NB * k + b
                w0 = wt[:, b, 0:1]
                w1 = wt[:, b, 1:2]
                w2 = wt[:, b, 2:3]
                y_ = ys[b]
                x_ = xs[b]
                # the single allowed attached wait rides on the block's
                # first compute op; semL counts the wtile load too, so
                # 16*(i+2) = wtile + loads 0..i complete
                ts = eng.tensor_scalar_mul(y_, x_[:, :, :], w1)
                ts.wait_op(semL, 16 * (i + 2), "sem-ge")
                for c in range(n_clips):
                    lo, hi = c * S, (c + 1) * S
                    eng.scalar_tensor_tensor(
                        y_[:, lo + 1:hi, :], x_[:, lo:hi - 1, :], w0,
                        y_[:, lo + 1:hi, :], MULT, ADD)
                    last = eng.scalar_tensor_tensor(
                        y_[:, lo:hi - 1, :], x_[:, lo + 1:hi, :], w2,
                        y_[:, lo:hi - 1, :], MULT, ADD)
                last.then_inc(semC, 1)

    return loads, stores, compute


def _build(F, C, HW, n_seg):
    """Single-pass program: x (F, C, HW) f32 -> out (F, C, HW) bf16."""
    nc = bacc.Bacc("TRN2", target_bir_lowering=False, debug=False,
                   num_devices=N_CORES)
    x = nc.dram_tensor("x", (F, C, HW), FP, kind="ExternalInput")
    w = nc.dram_tensor("weight", (C, 3), FP, kind="ExternalInput")
    out = nc.dram_tensor("out", (F, C, HW), BF, kind="ExternalOutput")
    NB = C // P

    stack = contextlib.ExitStack()
    block = stack.enter_context(nc.Block())
    sems = tuple(stack.enter_context(nc.semaphore(s))
                 for s in ("semF", "semL", "semC", "semS"))
    wt = stack.enter_context(nc.sbuf_tensor("wt", [P, NB, 3], FP))
    xA = stack.enter_context(nc.sbuf_tensor("xA", [P, F, HW], BF))
    xB = stack.enter_context(nc.sbuf_tensor("xB", [P, F, HW], BF))
    yb = stack.enter_context(nc.sbuf_tensor("yb", [P, F, NB, HW], BF))

    loads, stores, compute = emit_conv_raw(
        nc, w, sems, (wt, xA, xB, yb), lambda k: x, lambda k: out,
        F, C, n_seg, repeat=1, chain=False)

    block.gpsimd(loads)
    block.scalar(stores)
    block.vector(compute)
    stack.close()
    nc.compile()
    return nc


def _get_program(F, C, HW, n_seg):
    key = (F, C, HW, n_seg)
    if key not in _cache:
        _cache[key] = _build(F, C, HW, n_seg)
    return _cache[key]


def kernel(x, weight, n_segment, **_kw):
    x = np.asarray(x)
    weight = np.ascontiguousarray(np.asarray(weight, dtype=np.float32))
    n_seg = int(np.asarray(n_segment))
    nt, C, H, W = x.shape
    HW = H * W
    assert nt % N_CORES == 0
    F = nt // N_CORES
    # each core must hold whole clips
    assert F % n_seg == 0 or n_seg % F == 0, (F, n_seg)
    assert C % P == 0, C

    nc = _get_program(F, C, HW, n_seg)

    xs = np.ascontiguousarray(x, dtype=np.float32).reshape(nt, C, HW)
    in_maps = [
        {"x": xs[i * F:(i + 1) * F], "weight": weight} for i in range(N_CORES)
    ]
    res = run_bass_kernel_spmd(nc, in_maps, list(range(N_CORES)))
    out = np.concatenate(
        [np.asarray(res.results[i]["out"], dtype=np.float32)
         for i in range(N_CORES)], axis=0)
    return out.reshape(nt, C, H, W).astype(np.float32, copy=False)


# revision 10
# speedup vs baseline: 1.5692x; 1.1990x over previous
"""TSM-style 3-tap depthwise temporal conv on 8 Trainium2 NeuronCores.

out[n, t, c, h, w] = w[c,0]*x[n,t-1,c,h,w] + w[c,1]*x[n,t,c,h,w]
                   + w[c,2]*x[n,t+1,c,h,w]   (zero-padded at clip edges)

Sharding: pure data parallel over the nt (clip-batch) axis — each of the 8
cores gets whole clips (nt=64, n_segment=8 -> one 8-frame clip per core).
Weight (c,3) is replicated.

Platform model (measured on this axon/trn2 virtualized stack): execution is
dominated by a large per-instruction dispatch cost (~40-60us plus a
size-dependent part), with limited engine/DMA overlap; standalone semaphore
instructions cost as much as compute ops. Design consequences:

  - raw bacc (nc.Block) instead of the Tile framework: every semaphore inc
    is attached to a data instruction via .then_inc and every DVE wait rides
    on a compute instruction via .wait_op (the Tile scheduler emits ~4
    standalone EventSemaphore instructions per pass, each costing a full
    dispatch). Note: an instruction holds at most ONE attached wait, and
    every DMA must carry a sem update or walrus crashes.
  - minimal instruction count (10 per pass): 2 casting loads (fp32 DRAM ->
    bf16 SBUF via gpsimd SWDGE, 12.8MB read each) into SEPARATE x tiles so
    the block-B load overlaps block-A compute; 6 DVE ops (tensor_scalar_mul
    + 2 scalar_tensor_tensor per 128-channel block, all-bf16 operands with
    fp32 per-partition weight scalars); 2 per-block bf16 stores, so store A
    overlaps compute B and the steady-state cycle (compute -> store -> next
    load on the same buffers) is per-block rather than whole-pass.
  - bf16 x and y: rel err vs the fp32 reference is ~9e-3 (input rounding +
    3 output roundings), inside the 2e-2 gate; halves store bytes and
    speeds DVE accumulation.

Measured (k=2/122 repeat-chain differencing): ~480-580us/pass vs 697us for
the Tile-framework fp32 baseline.
"""

import contextlib

import numpy as np

import concourse.bacc as bacc
import concourse.mybir as mybir
from concourse.bass_utils import run_bass_kernel_spmd

N_CORES = 8
P = 128  # SBUF partitions

FP = mybir.dt.float32
BF = mybir.dt.bfloat16
MULT = mybir.AluOpType.mult
ADD = mybir.AluOpType.add

_cache = {}


def emit_conv_raw(nc, w, sems, tiles, src, dst, F, C, n_seg, repeat=1,
                  chain=False):
    """Emit bodies for `repeat` conv passes src->dst (raw bacc, bf16 x/y).

    Returns (loads_body, stores_body, compute_body) closures for the
    gpsimd / scalar / vector engines. src/dst: callables k -> DRAM handle
    (src fp32, dst bf16). tiles: (wt, xA, xB, yb). chain=True adds the
    cross-pass waits used by the timing harness's scratch chain; the real
    kernel uses repeat=1, chain=False.
    """
    semF, semL, semC, semS, semT = sems
    wt, xA, xB, yb = tiles
    NB = C // P
    HW = xA.shape[2]
    n_clips = max(F // n_seg, 1)
    S = min(n_seg, F)
    xs = [xA, xB]
    ys = [yb[:, :, b, :] for b in range(NB)]

    def src_view(k, b):
        return src(k)[:, b * P:(b + 1) * P, :].rearrange("f c x -> c f x")

    def loads(eng):
        # gpsimd SWDGE: fp32 DRAM -> bf16 SBUF casting loads
        eng.dma_start(wt[:, :, :],
                      w.ap().rearrange("(b c) k -> c b k", c=P)).then_inc(semL, 16)
        if chain:
            eng.wait_ge(semF, 16 * 2 * 16 + 1)  # scratch fill done
        for k in range(repeat):
            for b in range(NB):
                if chain and k > 0:
                    # x[b] WAR: compute (k-1, b) consumed it
                    eng.wait_ge(semC, NB * (k - 1) + b + 1)
                    # y[b] WAR: store (k-1, b) done reading this block's y;
                    # ts of pass k waits this load, which waits that store
                    # (a DVE instruction holds only one attached wait)
                    eng.wait_ge(semS, 16 * (NB * (k - 1) + b + 1))
                eng.dma_start(xs[b][:, :, :], src_view(k, b)).then_inc(semL, 16)

    def scalar_ops(eng):
        # ACT queue: per pass, the two tensor_scalar muls (y = w1*x) then the
        # two per-block stores. Offloading the muls halves DVE busy time;
        # store b=0 runs concurrently with block-1 compute, and next-pass
        # loads wait only their own block's store.
        for k in range(repeat):
            for b in range(NB):
                i = NB * k + b
                eng.wait_ge(semL, 16 * (i + 2))  # wtile + loads 0..i done
                eng.mul(ys[b], xs[b][:, :, :], wt[:, b, 1:2]).then_inc(semT, 1)
            for b in range(NB):
                eng.wait_ge(semC, NB * k + b + 1)
                eng.dma_start(
                    dst(k)[:, b * P:(b + 1) * P, :].rearrange("f c x -> c f x"),
                    ys[b]).then_inc(semS, 16)

    def compute(eng):
        # DVE: only the 4 accumulation taps; each block's first stt carries
        # the (single allowed) attached wait on the ACT mul for that block
        for k in range(repeat):
            for b in range(NB):
                i = NB * k + b
                w0 = wt[:, b, 0:1]
                w2 = wt[:, b, 2:3]
                y_ = ys[b]
                x_ = xs[b]
                first = True
                for c in range(n_clips):
                    lo, hi = c * S, (c + 1) * S
                    stt = eng.scalar_tensor_tensor(
                        y_[:, lo + 1:hi, :], x_[:, lo:hi - 1, :], w0,
                        y_[:, lo + 1:hi, :], MULT, ADD)
                    if first:
                        stt.wait_op(semT, i + 1, "sem-ge")
                        first = False
                    last = eng.scalar_tensor_tensor(
                        y_[:, lo:hi - 1, :], x_[:, lo + 1:hi, :], w2,
                        y_[:, lo:hi - 1, :], MULT, ADD)
                last.then_inc(semC, 1)

    return loads, scalar_ops, compute


def _build(F, C, HW, n_seg):
    """Single-pass program: x (F, C, HW) f32 -> out (F, C, HW) bf16."""
    nc = bacc.Bacc("TRN2", target_bir_lowering=False, debug=False,
                   num_devices=N_CORES)
    x = nc.dram_tensor("x", (F, C, HW), FP, kind="ExternalInput")
    w = nc.dram_tensor("weight", (C, 3), FP, kind="ExternalInput")
    out = nc.dram_tensor("out", (F, C, HW), BF, kind="ExternalOutput")
    NB = C // P

    stack = contextlib.ExitStack()
    block = stack.enter_context(nc.Block())
    sems = tuple(stack.enter_context(nc.semaphore(s))
                 for s in ("semF", "semL", "semC", "semS", "semT"))
    wt = stack.enter_context(nc.sbuf_tensor("wt", [P, NB, 3], FP))
    xA = stack.enter_context(nc.sbuf_tensor("xA", [P, F, HW], BF))
    xB = stack.enter_context(nc.sbuf_tensor("xB", [P, F, HW], BF))
    yb = stack.enter_context(nc.sbuf_tensor("yb", [P, F, NB, HW], BF))

    loads, stores, compute = emit_conv_raw(
        nc, w, sems, (wt, xA, xB, yb), lambda k: x, lambda k: out,
        F, C, n_seg, repeat=1, chain=False)

    block.gpsimd(loads)
    block.scalar(stores)
    block.vector(compute)
    stack.close()
    nc.compile()
    return nc


def _get_program(F, C, HW, n_seg):
    key = (F, C, HW, n_seg)
    if key not in _cache:
        _cache[key] = _build(F, C, HW, n_seg)
    return _cache[key]


def kernel(x, weight, n_segment, **_kw):
    x = np.asarray(x)
    weight = np.ascontiguousarray(np.asarray(weight, dtype=np.float32))
    n_seg = int(np.asarray(n_segment))
    nt, C, H, W = x.shape
    HW = H * W
    assert nt % N_CORES == 0
    F = nt // N_CORES
    # each core must hold whole clips
    assert F % n_seg == 0 or n_seg % F == 0, (F, n_seg)
    assert C % P == 0, C

    nc = _get_program(F, C, HW, n_seg)

    xs = np.ascontiguousarray(x, dtype=np.float32).reshape(nt, C, HW)
    in_maps = [
        {"x": xs[i * F:(i + 1) * F], "weight": weight} for i in range(N_CORES)
    ]
    res = run_bass_kernel_spmd(nc, in_maps, list(range(N_CORES)))
    out = np.concatenate(
        [np.asarray(res.results[i]["out"], dtype=np.float32)
         for i in range(N_CORES)], axis=0)
    return out.reshape(nt, C, H, W).astype(np.float32, copy=False)


# revision 12
# speedup vs baseline: 3.3371x; 2.1266x over previous
"""TSM-style 3-tap depthwise temporal conv on 8 Trainium2 NeuronCores.

out[n, t, c, h, w] = w[c,0]*x[n,t-1,c,h,w] + w[c,1]*x[n,t,c,h,w]
                   + w[c,2]*x[n,t+1,c,h,w]   (zero-padded at clip edges)

Sharding: pure data parallel over the nt (clip-batch) axis — each of the 8
cores gets whole clips (nt=64, n_segment=8 -> one 8-frame clip per core).
Weight (c,3) is replicated.

Platform model (measured on this axon/trn2 virtualized stack): execution is
dominated by a large per-instruction dispatch cost (~40-60us plus a
size-dependent part), with limited engine/DMA overlap; standalone semaphore
instructions cost as much as compute ops. Design consequences:

  - raw bacc (nc.Block) instead of the Tile framework: every semaphore inc
    is attached to a data instruction via .then_inc and every DVE wait rides
    on a compute instruction via .wait_op (the Tile scheduler emits ~4
    standalone EventSemaphore instructions per pass, each costing a full
    dispatch). Note: an instruction holds at most ONE attached wait, and
    every DMA must carry a sem update or walrus crashes.
  - minimal instruction count (10 per pass) spread over three engines for
    overlap: gpsimd runs 2 casting loads (fp32 DRAM -> bf16 SBUF SWDGE,
    12.8MB read each) into SEPARATE x tiles so the block-B load overlaps
    block-A compute; ACT runs the two tensor-scalar muls (y = w1*x) plus 2
    per-block bf16 stores, halving DVE busy time; DVE runs only the 4
    scalar_tensor_tensor accumulation taps. Per-block stores keep the
    steady-state cycle (compute -> store -> next load on the same buffers)
    per-block rather than whole-pass.
  - bf16 x and y: rel err vs the fp32 reference is ~9e-3 (input rounding +
    3 output roundings), inside the 2e-2 gate; halves store bytes and
    speeds DVE accumulation.

Measured (k=2/122 repeat-chain differencing): 207-375us/pass depending on
device warm state, vs 697us for the Tile-framework fp32 baseline.
"""

import contextlib

import numpy as np

import concourse.bacc as bacc
import concourse.mybir as mybir
from concourse.bass_utils import run_bass_kernel_spmd

N_CORES = 8
P = 128  # SBUF partitions

FP = mybir.dt.float32
BF = mybir.dt.bfloat16
MULT = mybir.AluOpType.mult
ADD = mybir.AluOpType.add

_cache = {}


def emit_conv_raw(nc, w, sems, tiles, src, dst, F, C, n_seg, repeat=1,
                  chain=False):
    """Emit bodies for `repeat` conv passes src->dst (raw bacc, bf16 x/y).

    Returns (loads_body, stores_body, compute_body) closures for the
    gpsimd / scalar / vector engines. src/dst: callables k -> DRAM handle
    (src fp32, dst bf16). tiles: (wt, xA, xB, yb). chain=True adds the
    cross-pass waits used by the timing harness's scratch chain; the real
    kernel uses repeat=1, chain=False.
    """
    semF, semL, semC, semS, semT = sems
    wt, xA, xB, yb = tiles
    NB = C // P
    HW = xA.shape[2]
    n_clips = max(F // n_seg, 1)
    S = min(n_seg, F)
    xs = [xA, xB]
    ys = [yb[:, :, b, :] for b in range(NB)]

    def src_view(k, b):
        return src(k)[:, b * P:(b + 1) * P, :].rearrange("f c x -> c f x")

    def loads(eng):
        # gpsimd SWDGE: fp32 DRAM -> bf16 SBUF casting loads
        eng.dma_start(wt[:, :, :],
                      w.ap().rearrange("(b c) k -> c b k", c=P)).then_inc(semL, 16)
        if chain:
            eng.wait_ge(semF, 16 * 2 * 16 + 1)  # scratch fill done
        for k in range(repeat):
            for b in range(NB):
                if chain and k > 0:
                    # x[b] WAR: compute (k-1, b) consumed it
                    eng.wait_ge(semC, NB * (k - 1) + b + 1)
                    # y[b] WAR: store (k-1, b) done reading this block's y;
                    # ts of pass k waits this load, which waits that store
                    # (a DVE instruction holds only one attached wait)
                    eng.wait_ge(semS, 16 * (NB * (k - 1) + b + 1))
                eng.dma_start(xs[b][:, :, :], src_view(k, b)).then_inc(semL, 16)

    def scalar_ops(eng):
        # ACT queue: only the two tensor_scalar muls (y = w1*x) per pass.
        # Offloading them halves DVE busy time.
        for k in range(repeat):
            for b in range(NB):
                i = NB * k + b
                eng.wait_ge(semL, 16 * (i + 2))  # wtile + loads 0..i done
                eng.mul(ys[b], xs[b][:, :, :], wt[:, b, 1:2]).then_inc(semT, 1)

    def stores(eng):
        # sync queue (otherwise idle): per-block stores. Store b=0 runs
        # concurrently with block-1 compute; next-pass loads wait only
        # their own block's store.
        for k in range(repeat):
            for b in range(NB):
                eng.wait_ge(semC, NB * k + b + 1)
                eng.dma_start(
                    dst(k)[:, b * P:(b + 1) * P, :].rearrange("f c x -> c f x"),
                    ys[b]).then_inc(semS, 16)

    def compute(eng):
        # DVE: only the 4 accumulation taps; each block's first stt carries
        # the (single allowed) attached wait on the ACT mul for that block
        for k in range(repeat):
            for b in range(NB):
                i = NB * k + b
                w0 = wt[:, b, 0:1]
                w2 = wt[:, b, 2:3]
                y_ = ys[b]
                x_ = xs[b]
                first = True
                for c in range(n_clips):
                    lo, hi = c * S, (c + 1) * S
                    stt = eng.scalar_tensor_tensor(
                        y_[:, lo + 1:hi, :], x_[:, lo:hi - 1, :], w0,
                        y_[:, lo + 1:hi, :], MULT, ADD)
                    if first:
                        stt.wait_op(semT, i + 1, "sem-ge")
                        first = False
                    last = eng.scalar_tensor_tensor(
                        y_[:, lo:hi - 1, :], x_[:, lo + 1:hi, :], w2,
                        y_[:, lo:hi - 1, :], MULT, ADD)
                last.then_inc(semC, 1)

    return loads, scalar_ops, stores, compute


def _build(F, C, HW, n_seg):
    """Single-pass program: x (F, C, HW) f32 -> out (F, C, HW) bf16."""
    nc = bacc.Bacc("TRN2", target_bir_lowering=False, debug=False,
                   num_devices=N_CORES)
    x = nc.dram_tensor("x", (F, C, HW), FP, kind="ExternalInput")
    w = nc.dram_tensor("weight", (C, 3), FP, kind="ExternalInput")
    out = nc.dram_tensor("out", (F, C, HW), BF, kind="ExternalOutput")
    NB = C // P

    stack = contextlib.ExitStack()
    block = stack.enter_context(nc.Block())
    sems = tuple(stack.enter_context(nc.semaphore(s))
                 for s in ("semF", "semL", "semC", "semS", "semT"))
    wt = stack.enter_context(nc.sbuf_tensor("wt", [P, NB, 3], FP))
    xA = stack.enter_context(nc.sbuf_tensor("xA", [P, F, HW], BF))
    xB = stack.enter_context(nc.sbuf_tensor("xB", [P, F, HW], BF))
    yb = stack.enter_context(nc.sbuf_tensor("yb", [P, F, NB, HW], BF))

    loads, scalar_ops, stores, compute = emit_conv_raw(
        nc, w, sems, (wt, xA, xB, yb), lambda k: x, lambda k: out,
        F, C, n_seg, repeat=1, chain=False)

    block.gpsimd(loads)
    block.scalar(scalar_ops)
    block.sync(stores)
    block.vector(compute)
    stack.close()
    nc.compile()
    return nc


def _get_program(F, C, HW, n_seg):
    key = (F, C, HW, n_seg)
    if key not in _cache:
        _cache[key] = _build(F, C, HW, n_seg)
    return _cache[key]


def kernel(x, weight, n_segment, **_kw):
    x = np.asarray(x)
    weight = np.ascontiguousarray(np.asarray(weight, dtype=np.float32))
    n_seg = int(np.asarray(n_segment))
    nt, C, H, W = x.shape
    HW = H * W
    assert nt % N_CORES == 0
    F = nt // N_CORES
    # each core must hold whole clips
    assert F % n_seg == 0 or n_seg % F == 0, (F, n_seg)
    assert C % P == 0, C

    nc = _get_program(F, C, HW, n_seg)

    xs = np.ascontiguousarray(x, dtype=np.float32).reshape(nt, C, HW)
    in_maps = [
        {"x": xs[i * F:(i + 1) * F], "weight": weight} for i in range(N_CORES)
    ]
    res = run_bass_kernel_spmd(nc, in_maps, list(range(N_CORES)))
    out = np.concatenate(
        [np.asarray(res.results[i]["out"], dtype=np.float32)
         for i in range(N_CORES)], axis=0)
    return out.reshape(nt, C, H, W).astype(np.float32, copy=False)
